# revision 1
# baseline (speedup 1.0000x reference)
"""Trainium2 Bass kernel: 16-head MHA (S=2048, D=1024, Dk=Dv=64) on 8 NeuronCores.

Sharding: tensor-parallel over heads — 2 heads per core (per the sharding
hint). Each core projects Q/K/V for its 2 heads, computes scores in
transposed layout S^T[t, s] = K_h Q_h^T (so the AV matmul can contract over
t on the partition axis), applies exp on the Scalar engine with the 1/sqrt(64)
scale fused in, and accumulates heads^T = V_aug^T @ exp(S^T) where V_aug has a
ones-column appended so the softmax denominator drops out of the same matmul
(PSUM row 64). Normalization multiplies by the broadcast reciprocal.

Final projection is row-sharded Wo: each core computes
  partial_out^T[c, s] = Wo[rows_of_its_heads].T @ heads^T  (+ bo on core 0)
and the unshard step sums the 8 partial outputs and transposes (row-parallel
linear layer; the reduce happens during unsharding).

Matmuls run in bf16 (inputs pre-rounded on host during sharding prep) with
fp32 PSUM accumulation; softmax statistics and the output stay fp32.
"""

import numpy as np

import concourse.tile as tile_mod
from concourse import bacc, mybir
from concourse.bass_utils import run_bass_kernel_spmd
from concourse.vector_clock import ScopedClock, VectorClock

F32 = mybir.dt.float32
BF16 = mybir.dt.bfloat16

S, D, H, DK = 2048, 1024, 16, 64
P = 128
NCORES = 8


def _patched_drain_and_barrier(self, tick_clock, wait_clock):
    """This container's walrus build caps CTRL-type instructions at one sem
    wait, but Tile's exit drain carries one wait per outstanding proc. Emit
    one Drain per outstanding proc instead, each with a single wait."""
    gc = tick_clock.global_clock
    vec = list(gc)
    for i, t in enumerate(vec):
        if t <= 0:
            continue
        pv = [0] * len(vec)
        pv[i] = t
        d = self.nc.sync.drain()
        wait_clock.add_sem_waits(d.ins, ScopedClock({None: VectorClock(pv)}))

    self.nc.all_engine_barrier()
    assert self.sems is not None
    popped = self.nc._tile_sem_poison_stack.pop()
    assert popped is self._sem_poison
    self.nc.clear_and_free_semaphores(list(self.sems.allocated().values()))
    self.nc.all_engine_barrier()


tile_mod.TileContext._drain_and_barrier = _patched_drain_and_barrier


def _build_nc():
    from contextlib import ExitStack

    tile = tile_mod
    nc = bacc.Bacc(None)

    et = nc.declare_dram_parameter("et", [D, S], BF16, isOutput=False)
    wqkv = nc.declare_dram_parameter("wqkv", [D, 6 * DK], BF16, isOutput=False)
    bqk = nc.declare_dram_parameter("bqk", [P, 2], F32, isOutput=False)
    bv = nc.declare_dram_parameter("bv", [P, 2 * DK], F32, isOutput=False)
    bo = nc.declare_dram_parameter("bo", [P, 8], F32, isOutput=False)
    wo = nc.declare_dram_parameter("wo", [P, D], BF16, isOutput=False)
    out = nc.declare_dram_parameter("out", [D, S], F32, isOutput=True)

    et3 = et.rearrange("(po pi) s -> pi po s", pi=P)      # [128, 8, 2048]
    wqkv3 = wqkv.rearrange("(po pi) c -> pi po c", pi=P)  # [128, 8, 384]

    with tile.TileContext(nc) as tc, ExitStack() as ctx:
        consts = ctx.enter_context(tc.tile_pool(name="consts", bufs=1))
        qkv = ctx.enter_context(tc.tile_pool(name="qkv", bufs=1))
        utp = ctx.enter_context(tc.tile_pool(name="ut", bufs=3))
        headsp = ctx.enter_context(tc.tile_pool(name="heads", bufs=2))
        normp = ctx.enter_context(tc.tile_pool(name="norm", bufs=4))
        outp = ctx.enter_context(tc.tile_pool(name="outp", bufs=3))
        psum = ctx.enter_context(tc.tile_pool(name="psum", bufs=1, space="PSUM"))
        dramsm = ctx.enter_context(tc.tile_pool(name="dramsm", bufs=4, space="DRAM"))

        # ---- load constants -------------------------------------------------
        # wqkv + biases + wo go on the ACT HWDGE ring so they are not stuck
        # FIFO behind the 4MB et transfer on the sync ring.
        wqkv_sb = consts.tile([P, 8, 6 * DK], BF16)
        nc.scalar.dma_start(wqkv_sb[:], wqkv3[:])
        bqk_sb = consts.tile([P, 2], F32)
        nc.scalar.dma_start(bqk_sb[:], bqk[:])
        bv_b = consts.tile([P, 2 * DK], F32)
        nc.scalar.dma_start(bv_b[:], bv[:])
        bo_c = consts.tile([P, 8], F32)
        nc.scalar.dma_start(bo_c[:], bo[:])
        et_sb = consts.tile([P, 8, S], BF16)
        nc.sync.dma_start(et_sb[:, 0:2, :], et3[:, 0:2, :])
        nc.scalar.dma_start(et_sb[:, 2:4, :], et3[:, 2:4, :])
        nc.sync.dma_start(et_sb[:, 4:6, :], et3[:, 4:6, :])
        nc.scalar.dma_start(et_sb[:, 6:8, :], et3[:, 6:8, :])

        # ---- QKV projections ------------------------------------------------
        qt_sb = qkv.tile([P, S], BF16)
        kt_sb = qkv.tile([P, S], BF16)
        vaug_sb = qkv.tile([P, 16, 130], BF16)
        nc.vector.memset(vaug_sb[:, :, 64:65], 1.0)
        nc.vector.memset(vaug_sb[:, :, 129:130], 1.0)

        # PSUM budget (8 banks): st [128,1024] x2 bufs = 4, av [65,1024] x1 = 2,
        # op [128,512] x2 = 2. QKV-phase psums rotate over all five slots.
        mm_tags = ["st", "st", "av", "op", "op"]
        mm_bufs = {"st": 2, "av": 1, "op": 2}
        tag_i = 0

        def next_tag():
            nonlocal tag_i
            t = mm_tags[tag_i % 5]
            tag_i += 1
            return t

        # Q^T / K^T: psum [128, 512] = sum_dc wqkv[:, dc, sel].T @ et[:, dc, sc]
        for sc in range(4):
            s0 = sc * 512
            for which, dst in ((0, qt_sb), (1, kt_sb)):
                tg = next_tag()
                ps = psum.tile([P, 512], F32, tag=tg, bufs=mm_bufs[tg])
                for dc in range(8):
                    nc.tensor.matmul(
                        ps[:],
                        wqkv_sb[:, dc, which * 128 : which * 128 + 128],
                        et_sb[:, dc, s0 : s0 + 512],
                        start=(dc == 0),
                        stop=(dc == 7),
                    )
                nc.vector.tensor_scalar_add(
                    dst[:, s0 : s0 + 512], ps[:], bqk_sb[:, which : which + 1]
                )

        # V natural [t, v]: psum = sum_dc et[:, dc, tb].T @ wqkv[:, dc, 256:384]
        for tb in range(16):
            t0 = tb * P
            tg = next_tag()
            ps = psum.tile([P, P], F32, tag=tg, bufs=mm_bufs[tg])
            for dc in range(8):
                nc.tensor.matmul(
                    ps[:],
                    et_sb[:, dc, t0 : t0 + P],
                    wqkv_sb[:, dc, 256:384],
                    start=(dc == 0),
                    stop=(dc == 7),
                )
            nc.vector.tensor_tensor(
                vaug_sb[:, tb, 0:64], ps[:, 0:64], bv_b[:, 0:64], mybir.AluOpType.add
            )
            nc.vector.tensor_tensor(
                vaug_sb[:, tb, 65:129], ps[:, 64:128], bv_b[:, 64:128],
                mybir.AluOpType.add,
            )

        # ---- attention + row-sharded output projection ----------------------
        wo_sb = consts.tile([P, D], BF16)
        nc.scalar.dma_start(wo_sb[:], wo[:])

        def emit_outproj(sh, heads_sb):
            # out^T[c, s] = wo_rows.T @ heads^T (+ bo as per-partition scalar).
            # sh=0 runs concurrently with attention (only the 2 "op" slots are
            # free); sh=1 runs after attention, so rotate over all 8 banks.
            rot = (
                [("op", 2)]
                if sh == 0
                else [("op", 2), ("op", 2), ("st", 2), ("st", 2), ("av", 1)]
            )
            for blk in range(8):
                c0 = blk * P
                for ch in range(2):
                    s0 = ch * 512
                    tg, bfs = rot[(blk * 2 + ch) % len(rot)]
                    ps = psum.tile(
                        [P, 512], F32, tag=tg, bufs=bfs, name=f"op_{sh}_{blk}_{ch}"
                    )
                    nc.tensor.matmul(
                        ps[:],
                        wo_sb[:, c0 : c0 + P],
                        heads_sb[:, s0 : s0 + 512],
                        start=True,
                        stop=True,
                    )
                    ot = outp.tile([P, 512], F32, tag="out")
                    if sh == 0 or (blk + ch) % 2 == 0:
                        nc.vector.tensor_scalar_add(
                            ot[:], ps[:], bo_c[:, blk : blk + 1]
                        )
                    else:
                        nc.scalar.activation(
                            ot[:],
                            ps[:],
                            mybir.ActivationFunctionType.Identity,
                            bias=bo_c[:, blk : blk + 1],
                        )
                    eng = nc.sync if (blk + ch) % 2 == 0 else nc.scalar
                    eng.dma_start(
                        out[c0 : c0 + P, sh * 1024 + s0 : sh * 1024 + s0 + 512],
                        ot[:],
                    )

        # Head-sequential attention passes: st is double-buffered across
        # t-blocks; av persists per pass; "op" slots stay free so the output
        # projection overlaps attention.
        for sh in range(2):
            h0 = sh * 1024
            heads_sb = headsp.tile([P, 1024], BF16, tag="heads", name=f"heads{sh}")
            for h in range(2):
                hp = h * 64
                av = psum.tile([65, 1024], F32, tag="av", bufs=1, name=f"av{sh}{h}")
                for tb in range(16):
                    t0 = tb * P
                    st = psum.tile(
                        [P, 1024], F32, tag="st", bufs=2, name=f"st{sh}{h}{tb}"
                    )
                    for n0 in (0, 512):
                        nc.tensor.matmul(
                            st[:, n0 : n0 + 512],
                            kt_sb[hp : hp + 64, t0 : t0 + P],
                            qt_sb[hp : hp + 64, h0 + n0 : h0 + n0 + 512],
                            start=True,
                            stop=True,
                        )
                    ut = utp.tile([P, 1024], BF16, tag="ut", bufs=4)
                    nc.scalar.activation(
                        ut[:], st[:], mybir.ActivationFunctionType.Exp, scale=0.125
                    )
                    for n0 in (0, 512):
                        nc.tensor.matmul(
                            av[:, n0 : n0 + 512],
                            vaug_sb[:, tb, h * 65 : h * 65 + 65],
                            ut[:, n0 : n0 + 512],
                            start=(tb == 0),
                            stop=(tb == 15),
                            skip_group_check=True,
                        )
                # Evacuate the AV psum immediately (frees the av slot for the
                # next pass), then normalize from SBUF: the softmax denominator
                # sits in row 64; reshape it across 128 partitions so the
                # reciprocal runs wide, then broadcast it back over v-rows.
                unnorm_sb = headsp.tile(
                    [64, 1024], F32, tag="unnorm", name=f"un{sh}{h}"
                )
                nc.vector.tensor_copy(unnorm_sb[:], av[0:64, :])
                dsb = normp.tile([1, 1024], F32, tag="denom_sb", name=f"dsb{sh}{h}")
                nc.vector.tensor_copy(dsb[:], av[64:65, :])
                rsh = normp.tile([P, 8], F32, tag="rsh")
                nc.sync.dma_start(rsh[:], dsb[:])
                nc.vector.reciprocal(rsh[:], rsh[:])
                recip_d = dramsm.tile([1, 1024], F32, tag="recip_d")
                nc.sync.dma_start(
                    recip_d.rearrange("o (p f) -> (o p) f", p=P), rsh[:]
                )
                recip_b = normp.tile([64, 1024], F32, tag="recip_b", name=f"rb{sh}{h}")
                nc.sync.dma_start(
                    recip_b[:], recip_d[0:1, :].to_broadcast((64, 1024))
                )
                nc.vector.tensor_tensor(
                    heads_sb[hp : hp + 64, :],
                    unnorm_sb[:],
                    recip_b[:],
                    mybir.AluOpType.mult,
                )
            emit_outproj(sh, heads_sb)

    nc.finalize()
    return nc


_NC_CACHE = None


def _get_nc():
    global _NC_CACHE
    if _NC_CACHE is None:
        _NC_CACHE = _build_nc()
    return _NC_CACHE


def _make_in_maps(embeddings, Wq, bq, Wk, bk, Wv, bv, Wo, bo):
    import ml_dtypes

    bf16 = np.dtype(ml_dtypes.bfloat16)
    et = np.ascontiguousarray(embeddings.T.astype(bf16))  # [1024, 2048]
    in_maps = []
    for c in range(NCORES):
        hs = [2 * c, 2 * c + 1]
        wqkv = np.concatenate(
            [Wq[hs[0]], Wq[hs[1]], Wk[hs[0]], Wk[hs[1]], Wv[hs[0]], Wv[hs[1]]],
            axis=1,
        ).astype(bf16)  # [1024, 384]
        bqk = np.stack(
            [np.concatenate([bq[hs[0]], bq[hs[1]]]),
             np.concatenate([bk[hs[0]], bk[hs[1]]])],
            axis=1,
        ).astype(np.float32)  # [128, 2]
        bvc = np.ascontiguousarray(
            np.broadcast_to(
                np.concatenate([bv[hs[0]], bv[hs[1]]])[None, :], (P, 2 * DK)
            ),
            dtype=np.float32,
        )
        bo_eff = bo if c == 0 else np.zeros_like(bo)
        in_maps.append(
            {
                "et": et,
                "wqkv": np.ascontiguousarray(wqkv),
                "bqk": np.ascontiguousarray(bqk),
                "bv": bvc,
                "bo": np.ascontiguousarray(bo_eff.reshape(8, P).T, dtype=np.float32),
                "wo": np.ascontiguousarray(Wo[c * P : (c + 1) * P].astype(bf16)),
            }
        )
    return in_maps


def kernel(embeddings, Wq, bq, Wk, bk, Wv, bv, Wo, bo, **run_kwargs):
    """Full-input / full-output MHA. Shards across 8 NeuronCores internally."""
    nc = _get_nc()
    in_maps = _make_in_maps(
        np.asarray(embeddings, np.float32),
        np.asarray(Wq, np.float32),
        np.asarray(bq, np.float32),
        np.asarray(Wk, np.float32),
        np.asarray(bk, np.float32),
        np.asarray(Wv, np.float32),
        np.asarray(bv, np.float32),
        np.asarray(Wo, np.float32),
        np.asarray(bo, np.float32),
    )
    res = run_bass_kernel_spmd(nc, in_maps, list(range(NCORES)), **run_kwargs)
    # Unshard the row-parallel output projection: sum the per-core partials
    # (each core contributed its 2 heads through its 128 rows of Wo), then
    # undo the on-chip out^T layout.
    acc = res.results[0]["out"].copy()
    for r_ in res.results[1:]:
        acc += r_["out"]
    return np.ascontiguousarray(acc.T)


if __name__ == "__main__":
    rng = np.random.default_rng(0)
    emb = rng.standard_normal((S, D), dtype=np.float32)
    mk = lambda *sh: (rng.standard_normal(sh, dtype=np.float32) * 0.02)
    o = kernel(
        embeddings=emb,
        Wq=mk(H, D, DK), bq=mk(H, DK),
        Wk=mk(H, D, DK), bk=mk(H, DK),
        Wv=mk(H, D, DK), bv=mk(H, DK),
        Wo=mk(H * DK, D), bo=mk(D),
    )
    print(o.shape, o.dtype)

